# revision 9
# baseline (speedup 1.0000x reference)
"""Decorrelation (whitening) normalization on 8 Trainium2 NeuronCores.

Reference semantics (NHWC input x: [32, 64, 64, 256], fp32):
  flatten to [N=131072, C=256]; per-channel mean; per-group (16 groups of
  m=16 channels) covariance with shrinkage ((1-eps)*cov + eps*I)/denom
  (cov itself already divided by denom once); L = cholesky(cov);
  out = L^{-1} (x - mean) per group, back in NHWC layout.

Strategy (data-parallel over batch, 8 cores, full I/O):
  - each core takes a contiguous 16384-row slice of [131072, 256]
  - phase 1: stream 128-sample chunks; accumulate full Gram (augmented with
    a ones column -> channel sums) via f32r matmuls in PSUM; PE-transpose
    each chunk into a persistent channel-major SBUF copy of the slice
  - tiny AllReduce (17 KB) of block-diag Gram + sums across the 8 cores
  - on-device per-group LDL elimination on [A | I] -> L^{-1} = D^{-1/2} M
  - phase 2: whitening matmuls (x_cm chunk stationary, block-diag W moving)
    produce sample-major output in PSUM; bias (W @ mean) subtract fused into
    the PSUM->SBUF copy; contiguous DMA back to HBM.
"""

import numpy as np

import concourse.bacc as bacc
import concourse.mybir as mybir
from concourse import masks, tile
from concourse.bass_types import AP
from concourse.bass_utils import run_bass_kernel_spmd

NCORES = 8
B, W_DIM, H_DIM, C = 32, 64, 64, 256
N_TOT = B * W_DIM * H_DIM            # 131072 samples (global)
NL = N_TOT // NCORES                 # 16384 samples per core
G = 16                               # channel groups
M = 16                               # channels per group
EPS = 1e-3
DENOM = float(N_TOT - 1)

CHUNK = 128                          # samples per PE chunk
CHUNKS = NL // CHUNK                 # 128
JPER = 4                             # chunks per staging tile / DMA
STAGES = CHUNKS // JPER              # 32

F32 = mybir.dt.float32
F32R = mybir.dt.float32r

# cov_final = A_COEF * Gram_raw - (A_COEF / N_TOT) * (S x S) + G_COEF * I
A_COEF = (1.0 - EPS) / (DENOM * DENOM)
G_COEF = EPS / DENOM

_CACHE = {}


def _build():
    nc = bacc.Bacc(
        "TRN2", target_bir_lowering=False, debug=False, num_devices=NCORES
    )
    x = nc.dram_tensor("x", [NL, C], F32, kind="ExternalInput").ap()
    o = nc.dram_tensor("o", [NL, C], F32, kind="ExternalOutput").ap()

    with tile.TileContext(nc) as tc:
        with (
            tc.tile_pool(name="const", bufs=1) as cpool,
            tc.tile_pool(name="persist", bufs=1) as ppool,
            tc.tile_pool(name="dram", bufs=1, space="DRAM") as dpool,
        ):
            ident = cpool.tile([128, 128], F32)
            masks.make_identity(nc, ident[:, :])
            ones1 = cpool.tile([1, 128], F32)
            nc.gpsimd.memset(ones1[:, :], 1.0)

            # channel-major copy of the local slice: half h holds channels
            # [128h, 128h+128) as [128, NL]
            x_cm0 = ppool.tile([128, NL], F32)
            x_cm1 = ppool.tile([128, NL], F32)
            # stats row p: [0:256] Gram row of ch p, [256] sum ch p,
            # [257:513] Gram row of ch 128+p, [513] sum ch 128+p
            stats = ppool.tile([128, 514], F32)
            cc_in = dpool.tile([128, 514], F32)
            cc_out = dpool.tile([128, 514], F32)
            w_dram = dpool.tile([16, 256], F32)
            b_dram = dpool.tile([1, 256], F32)

            # ---------------- phase 1: Gram + sums + transpose ----------
            with (
                tc.tile_pool(name="stage", bufs=3) as spool,
                tc.tile_pool(name="gpsum", bufs=1, space="PSUM") as gpool,
                tc.tile_pool(name="ptpsum", bufs=2, space="PSUM") as ptpool,
            ):
                g_top = gpool.tile([128, 257], F32, tag="gt")
                g_bot = gpool.tile([128, 257], F32, tag="gb")
                for t in range(STAGES):
                    st = spool.tile([128, JPER * 257], F32, tag="st")
                    st3 = st[:, :].rearrange("p (j c) -> p j c", j=JPER)
                    src = x[t * JPER * CHUNK:(t + 1) * JPER * CHUNK, :]
                    src = src.rearrange("(j p) c -> p j c", j=JPER)
                    nc.sync.dma_start(st3[:, :, 0:256], src)
                    nc.gpsimd.memset(st3[:, :, 256], 1.0)
                    for j in range(JPER):
                        ci = t * JPER + j
                        chunk = st3[:, j, :]
                        first = ci == 0
                        last = ci == CHUNKS - 1
                        mov = chunk[:, 0:257]
                        nc.tensor.matmul(
                            g_top[:, :], chunk[:, 0:128], mov,
                            start=first, stop=last,
                        )
                        nc.tensor.matmul(
                            g_bot[:, :], chunk[:, 128:256], mov,
                            start=first, stop=last,
                        )
                        pt0 = ptpool.tile([128, 128], F32, tag="pt0")
                        pt1 = ptpool.tile([128, 128], F32, tag="pt1")
                        nc.tensor.transpose(pt0[:, :], chunk[:, 0:128], ident[:, :])
                        nc.tensor.transpose(pt1[:, :], chunk[:, 128:256], ident[:, :])
                        cs = slice(ci * CHUNK, (ci + 1) * CHUNK)
                        nc.vector.tensor_copy(x_cm0[:, cs], pt0[:, :])
                        nc.vector.tensor_copy(x_cm1[:, cs], pt1[:, :])

                # ship full Gram rows + sums (engine reads must be
                # 32-partition aligned, so no block extraction here)
                nc.vector.tensor_copy(stats[:, 0:257], g_top[:, :])
                nc.vector.tensor_copy(stats[:, 257:514], g_bot[:, :])

            # ---------------- all-reduce the stats ----------------------
            nc.sync.dma_start(cc_in[:, :], stats[:, :])
            nc.gpsimd.collective_compute(
                "AllReduce",
                mybir.AluOpType.add,
                replica_groups=[list(range(NCORES))],
                ins=[cc_in.opt()],
                outs=[cc_out.opt()],
            )

            # ---------------- tiny solve: W = L^{-1} per group ----------
            # Aaug [16 groups (part), 16 rows x (16 A cols + 16 M cols)]
            aaug = ppool.tile([16, 512], F32)
            s_gl = ppool.tile([16, 16], F32)
            outer = ppool.tile([16, 256], F32)
            mu_sb = ppool.tile([128, 2], F32)
            w_gl = ppool.tile([16, 256], F32)
            w_bd0 = ppool.tile([128, 128], F32)
            w_bd1 = ppool.tile([128, 128], F32)
            b_sb = ppool.tile([128, 2], F32)
            b_row = ppool.tile([1, 256], F32)
            bias_rep = ppool.tile([128, 256], F32)

            a3 = aaug[:, :].rearrange("p (i c) -> p i c", i=16)   # [16,16,32]
            a_part = a3[:, :, 0:16]
            m_part = a3[:, :, 16:32]
            a_diag = aaug[:, 0:512:33]                            # A[i,i]
            m_diag = aaug[:, 16:512:33]                           # M[i,i]

            # gather per-group 16x16 Gram blocks + sums out of the
            # all-reduced full rows (custom strided APs on the DRAM tile;
            # flat(p, c) = 514 p + c)
            cc_t = cc_out[:, :].tensor
            for h in range(2):
                # block (g,i,k) of half h: flat = 514*(16g+i) + 16g+k + 385h
                src = AP(cc_t, 385 * h, [[8240, 8], [514, 16], [1, 16]])
                nc.sync.dma_start(a3[8 * h:8 * h + 8, :, 0:16], src)
                # sums (g,m): flat = 514*(16g+m) + 256 + 257h
                src_s = AP(cc_t, 256 + 257 * h, [[8224, 8], [514, 16]])
                nc.sync.dma_start(s_gl[8 * h:8 * h + 8, :], src_s)
            # mu[p, h] = S[128h + p] / N: flat = 514 p + 256 + 257 h
            nc.sync.dma_start(
                mu_sb[:, :], AP(cc_t, 256, [[514, 128], [257, 2]])
            )
            nc.vector.tensor_scalar_mul(mu_sb[:, :], mu_sb[:, :], 1.0 / N_TOT)

            # cov = A_COEF*Gram - (A_COEF/N)*S⊗S + G_COEF*I
            o3 = outer[:, :].rearrange("p (i k) -> p i k", i=16)
            nc.vector.tensor_tensor(
                o3,
                s_gl[:, :].unsqueeze(2).broadcast_to([16, 16, 16]),
                s_gl[:, :].unsqueeze(1).broadcast_to([16, 16, 16]),
                mybir.AluOpType.mult,
            )
            nc.vector.tensor_scalar_mul(o3, o3, A_COEF / N_TOT)
            nc.vector.tensor_scalar_mul(a_part, a_part, A_COEF)
            nc.vector.tensor_tensor(a_part, a_part, o3, mybir.AluOpType.subtract)
            nc.vector.tensor_scalar_add(a_diag, a_diag, G_COEF)
            # M := I
            nc.vector.memset(m_part, 0.0)
            nc.vector.memset(m_diag, 1.0)

            # gaussian elimination on [A | M]; row k eliminates rows k+1..15
            with tc.tile_pool(name="elim", bufs=2) as epool:
                for k in range(15):
                    nr = 15 - k          # rows below pivot
                    ncols = 31 - k       # remaining cols (A: k+1..15, M: 16)
                    rinv = epool.tile([16, 1], F32, tag="rinv")
                    nc.vector.reciprocal(rinv[:, :], aaug[:, 33 * k:33 * k + 1])
                    mcol = epool.tile([16, 16], F32, tag="mcol")
                    acol = aaug[:, 32 * (k + 1) + k:512:32]      # [16, nr]
                    nc.vector.tensor_scalar_mul(mcol[:, 0:nr], acol, rinv[:, :])
                    tmp = epool.tile([16, 16 * 32], F32, tag="etmp")
                    t3 = tmp[:, 0:nr * ncols].rearrange("p (j c) -> p j c", j=nr)
                    u = mcol[:, 0:nr].unsqueeze(2).broadcast_to([16, nr, ncols])
                    v = aaug[:, 32 * k + k + 1:32 * k + 32]
                    v = v.unsqueeze(1).broadcast_to([16, nr, ncols])
                    nc.vector.tensor_tensor(t3, u, v, mybir.AluOpType.mult)
                    blk = aaug[:, 32 * (k + 1):512]
                    blk = blk.rearrange("p (j c) -> p j c", c=32)[:, :, k + 1:32]
                    nc.vector.tensor_tensor(blk, blk, t3, mybir.AluOpType.subtract)

                # W rows: W[i, :] = sqrt(1/U[i,i]) * M[i, :]
                d16 = epool.tile([16, 16], F32, tag="d16")
                rsq = epool.tile([16, 16], F32, tag="rsq")
                nc.vector.reciprocal(d16[:, :], a_diag)
                nc.scalar.sqrt(rsq[:, :], d16[:, :])
                w3 = w_gl[:, :].rearrange("p (i k) -> p i k", i=16)
                nc.vector.tensor_tensor(
                    w3, m_part,
                    rsq[:, :].unsqueeze(2).broadcast_to([16, 16, 16]),
                    mybir.AluOpType.mult,
                )

            # scatter W into block-diagonal [128,128] halves (transposed:
            # W_bd[k_in, c_out] = W[c_out_local, k_local])
            nc.gpsimd.memset(w_bd0[:, :], 0.0)
            nc.gpsimd.memset(w_bd1[:, :], 0.0)
            nc.sync.dma_start(w_dram[:, :], w_gl[:, :])
            for g in range(G):
                h, gg = divmod(g, 8)
                dst_t = w_bd0 if h == 0 else w_bd1
                dst = dst_t[16 * gg:16 * gg + 16, 16 * gg:16 * gg + 16]
                src = w_dram[g].rearrange("(m k) -> k m", m=16)
                nc.sync.dma_start(dst, src)

            # bias = W_bd^T mean (per output channel), replicated to all rows
            with tc.tile_pool(name="bpsum", bufs=1, space="PSUM") as bpool:
                b0 = bpool.tile([128, 1], F32, tag="b0")
                b1 = bpool.tile([128, 1], F32, tag="b1")
                brep = bpool.tile([128, 256], F32, tag="brep")
                nc.tensor.matmul(b0[:, :], w_bd0[:, :], mu_sb[:, 0:1])
                nc.tensor.matmul(b1[:, :], w_bd1[:, :], mu_sb[:, 1:2])
                nc.vector.tensor_copy(b_sb[:, 0:1], b0[:, :])
                nc.vector.tensor_copy(b_sb[:, 1:2], b1[:, :])
                dst = b_dram[0].rearrange("(h p) -> p h", h=2)
                nc.sync.dma_start(dst, b_sb[:, :])
                nc.sync.dma_start(b_row[0:1, :], b_dram[0:1, :])
                nc.tensor.matmul(brep[:, :], ones1[0:1, :], b_row[0:1, :])
                nc.vector.tensor_copy(bias_rep[:, :], brep[:, :])

            # ---------------- phase 2: whiten + writeback ---------------
            with (
                tc.tile_pool(name="ostage", bufs=3) as opool,
                tc.tile_pool(name="popsum", bufs=3, space="PSUM") as popool,
            ):
                bias_v = bias_rep[:, :].rearrange("p (h c) -> p h c", h=2)
                for t in range(STAGES):
                    ost = opool.tile([128, JPER * 256], F32, tag="ost")
                    ost3 = ost[:, :].rearrange("p (j c) -> p j c", j=JPER)
                    for j in range(JPER):
                        ci = t * JPER + j
                        cs = slice(ci * CHUNK, (ci + 1) * CHUNK)
                        po = popool.tile([128, 1024], F32, tag="po")
                        nc.tensor.matmul(po[:, 0:128], x_cm0[:, cs], w_bd0[:, :])
                        nc.tensor.matmul(po[:, 512:640], x_cm1[:, cs], w_bd1[:, :])
                        po_v = po[:, :].rearrange("p (h c) -> p h c", h=2)
                        po_v = po_v[:, :, 0:128]
                        dst = ost3[:, j, :].rearrange("p (h c) -> p h c", h=2)
                        nc.vector.tensor_tensor(
                            dst, po_v, bias_v, mybir.AluOpType.subtract
                        )
                    dsto = o[t * JPER * CHUNK:(t + 1) * JPER * CHUNK, :]
                    dsto = dsto.rearrange("(j p) c -> p j c", j=JPER)
                    nc.sync.dma_start(dsto, ost3[:, :, :])

    nc.compile()
    return nc


def kernel(x: np.ndarray) -> np.ndarray:
    assert x.shape == (B, W_DIM, H_DIM, C) and x.dtype == np.float32
    if "nc" not in _CACHE:
        _CACHE["nc"] = _build()
    nc = _CACHE["nc"]
    xf = np.ascontiguousarray(x.reshape(N_TOT, C))
    in_maps = [
        {"x": xf[i * NL:(i + 1) * NL]} for i in range(NCORES)
    ]
    res = run_bass_kernel_spmd(nc, in_maps, list(range(NCORES)))
    out = np.concatenate([res.results[i]["o"] for i in range(NCORES)], axis=0)
    return out.reshape(B, W_DIM, H_DIM, C)


# revision 16
# speedup vs baseline: 1.2922x; 1.2922x over previous
"""Decorrelation (whitening) normalization on 8 Trainium2 NeuronCores.

Reference semantics (NHWC input x: [32, 64, 64, 256], fp32):
  flatten to [N=131072, C=256]; per-channel mean; per-group (16 groups of
  m=16 channels) covariance with shrinkage ((1-eps)*cov + eps*I)/denom
  (cov itself already divided by denom once); L = cholesky(cov);
  out = L^{-1} (x - mean) per group, back in NHWC layout.

Strategy (data-parallel over batch, 8 cores, full I/O):
  - each core takes a contiguous 16384-row slice of [131072, 256]
  - phase 1: stream 128-sample chunks; accumulate full Gram (augmented with
    a ones column -> channel sums) via f32r matmuls in PSUM; PE-transpose
    each chunk into a persistent channel-major SBUF copy of the slice
  - tiny AllReduce (17 KB) of block-diag Gram + sums across the 8 cores
  - on-device per-group LDL elimination on [A | I] -> L^{-1} = D^{-1/2} M
  - phase 2: whitening matmuls (x_cm chunk stationary, block-diag W moving)
    produce sample-major output in PSUM; bias (W @ mean) subtract fused into
    the PSUM->SBUF copy; contiguous DMA back to HBM.
"""

import numpy as np

import concourse.bacc as bacc
import concourse.mybir as mybir
from concourse import masks, tile
from concourse.bass_types import AP
from concourse.bass_utils import run_bass_kernel_spmd

NCORES = 8
B, W_DIM, H_DIM, C = 32, 64, 64, 256
N_TOT = B * W_DIM * H_DIM            # 131072 samples (global)
NL = N_TOT // NCORES                 # 16384 samples per core
G = 16                               # channel groups
M = 16                               # channels per group
EPS = 1e-3
DENOM = float(N_TOT - 1)

CHUNK = 128                          # samples per PE chunk
CHUNKS = NL // CHUNK                 # 128
JPER = 4                             # chunks per staging tile / DMA
STAGES = CHUNKS // JPER              # 32

F32 = mybir.dt.float32
BF16 = mybir.dt.bfloat16

# cov_final = A_COEF * Gram_raw - (A_COEF / N_TOT) * (S x S) + G_COEF * I
A_COEF = (1.0 - EPS) / (DENOM * DENOM)
G_COEF = EPS / DENOM

_CACHE = {}


def _build():
    nc = bacc.Bacc(
        "TRN2", target_bir_lowering=False, debug=False, num_devices=NCORES
    )
    x = nc.dram_tensor("x", [NL, C], F32, kind="ExternalInput").ap()
    o = nc.dram_tensor("o", [NL, C], F32, kind="ExternalOutput").ap()

    with tile.TileContext(nc) as tc:
        with (
            tc.tile_pool(name="const", bufs=1) as cpool,
            tc.tile_pool(name="persist", bufs=1) as ppool,
            tc.tile_pool(name="dram", bufs=1, space="DRAM") as dpool,
        ):
            ident = cpool.tile([128, 128], BF16)
            masks.make_identity(nc, ident[:, :])
            ones1 = cpool.tile([1, 128], F32)
            nc.gpsimd.memset(ones1[:, :], 1.0)

            # channel-major bf16 copy of the local slice: half h holds
            # channels [128h, 128h+128) as [128, NL]
            x_cm0 = ppool.tile([128, NL], BF16)
            x_cm1 = ppool.tile([128, NL], BF16)
            # stats row p: [0:256] Gram row of ch p, [256] sum ch p,
            # [257:513] Gram row of ch 128+p, [513] sum ch 128+p
            stats = ppool.tile([128, 514], F32)
            cc_in = dpool.tile([128, 514], F32)
            cc_out = dpool.tile([128, 514], F32)
            w_dram = dpool.tile([16, 256], F32)
            b_dram = dpool.tile([1, 256], F32)

            # ---------------- phase 1: Gram + sums + transpose ----------
            with (
                tc.tile_pool(name="stage", bufs=3) as spool,
                tc.tile_pool(name="gpsum", bufs=1, space="PSUM") as gpool,
                tc.tile_pool(name="ptpsum", bufs=2, space="PSUM") as ptpool,
            ):
                g_top = gpool.tile([128, 257], F32, tag="gt")
                g_bot = gpool.tile([128, 257], F32, tag="gb")
                for t in range(STAGES):
                    # SWDGE casting DMA: fp32 HBM -> bf16 SBUF in flight
                    st = spool.tile([128, JPER * 257], BF16, tag="st")
                    st3 = st[:, :].rearrange("p (j c) -> p j c", j=JPER)
                    src = x[t * JPER * CHUNK:(t + 1) * JPER * CHUNK, :]
                    src = src.rearrange("(j p) c -> p j c", j=JPER)
                    nc.gpsimd.dma_start(st3[:, :, 0:256], src)
                    nc.gpsimd.memset(st3[:, :, 256], 1.0)
                    for j in range(JPER):
                        ci = t * JPER + j
                        chunk = st3[:, j, :]
                        first = ci == 0
                        last = ci == CHUNKS - 1
                        mov = chunk[:, 0:257]
                        nc.tensor.matmul(
                            g_top[:, :], chunk[:, 0:128], mov,
                            start=first, stop=last,
                        )
                        nc.tensor.matmul(
                            g_bot[:, :], chunk[:, 128:256], mov,
                            start=first, stop=last,
                        )
                        pt0 = ptpool.tile([128, 128], BF16, tag="pt0")
                        pt1 = ptpool.tile([128, 128], BF16, tag="pt1")
                        nc.tensor.transpose(pt0[:, :], chunk[:, 0:128], ident[:, :])
                        nc.tensor.transpose(pt1[:, :], chunk[:, 128:256], ident[:, :])
                        cs = slice(ci * CHUNK, (ci + 1) * CHUNK)
                        nc.vector.tensor_copy(x_cm0[:, cs], pt0[:, :])
                        nc.vector.tensor_copy(x_cm1[:, cs], pt1[:, :])

                # ship full Gram rows + sums (engine reads must be
                # 32-partition aligned, so no block extraction here)
                nc.vector.tensor_copy(stats[:, 0:257], g_top[:, :])
                nc.vector.tensor_copy(stats[:, 257:514], g_bot[:, :])

            # ---------------- all-reduce the stats ----------------------
            nc.sync.dma_start(cc_in[:, :], stats[:, :])
            nc.gpsimd.collective_compute(
                "AllReduce",
                mybir.AluOpType.add,
                replica_groups=[list(range(NCORES))],
                ins=[cc_in.opt()],
                outs=[cc_out.opt()],
            )

            # ---------------- tiny solve: W = L^{-1} per group ----------
            # Aaug [16 groups (part), 16 rows x (16 A cols + 16 M cols)]
            aaug = ppool.tile([16, 512], F32)
            s_gl = ppool.tile([16, 16], F32)
            mu_gl = ppool.tile([16, 16], F32)
            outer = ppool.tile([16, 256], F32)
            wmu = ppool.tile([16, 256], F32)
            b_gl = ppool.tile([16, 16], F32)
            w_gl = ppool.tile([16, 256], F32)
            w_bd0 = ppool.tile([128, 128], BF16)
            w_bd1 = ppool.tile([128, 128], BF16)
            b_row = ppool.tile([1, 256], F32)
            bias_rep = ppool.tile([128, 256], F32)

            a3 = aaug[:, :].rearrange("p (i c) -> p i c", i=16)   # [16,16,32]
            a_part = a3[:, :, 0:16]
            m_part = a3[:, :, 16:32]
            a_diag = aaug[:, 0:512:33]                            # A[i,i]
            m_diag = aaug[:, 16:512:33]                           # M[i,i]

            # gather per-group 16x16 Gram blocks + sums out of the
            # all-reduced full rows (custom strided APs on the DRAM tile;
            # flat(p, c) = 514 p + c)
            cc_t = cc_out[:, :].tensor
            for h in range(2):
                # block (g,i,k) of half h: flat = 514*(16g+i) + 16g+k + 385h
                src = AP(cc_t, 385 * h, [[8240, 8], [514, 16], [1, 16]])
                nc.sync.dma_start(a3[8 * h:8 * h + 8, :, 0:16], src)
                # sums (g,m): flat = 514*(16g+m) + 256 + 257h
                src_s = AP(cc_t, 256 + 257 * h, [[8224, 8], [514, 16]])
                nc.sync.dma_start(s_gl[8 * h:8 * h + 8, :], src_s)
            nc.vector.tensor_scalar_mul(mu_gl[:, :], s_gl[:, :], 1.0 / N_TOT)

            # cov = A_COEF*Gram - (A_COEF/N)*S⊗S + G_COEF*I
            o3 = outer[:, :].rearrange("p (i k) -> p i k", i=16)
            nc.vector.tensor_tensor(
                o3,
                s_gl[:, :].unsqueeze(2).broadcast_to([16, 16, 16]),
                s_gl[:, :].unsqueeze(1).broadcast_to([16, 16, 16]),
                mybir.AluOpType.mult,
            )
            nc.vector.tensor_scalar_mul(o3, o3, A_COEF / N_TOT)
            nc.vector.tensor_scalar_mul(a_part, a_part, A_COEF)
            nc.vector.tensor_tensor(a_part, a_part, o3, mybir.AluOpType.subtract)
            nc.vector.tensor_scalar_add(a_diag, a_diag, G_COEF)
            # M := I
            nc.vector.memset(m_part, 0.0)
            nc.vector.memset(m_diag, 1.0)

            # gaussian elimination on [A | M]; row k eliminates rows k+1..15
            with tc.tile_pool(name="elim", bufs=2) as epool:
                for k in range(15):
                    nr = 15 - k          # rows below pivot
                    ncols = 31 - k       # remaining cols (A: k+1..15, M: 16)
                    rinv = epool.tile([16, 1], F32, tag="rinv")
                    nc.vector.reciprocal(rinv[:, :], aaug[:, 33 * k:33 * k + 1])
                    mcol = epool.tile([16, 16], F32, tag="mcol")
                    acol = aaug[:, 32 * (k + 1) + k:512:32]      # [16, nr]
                    nc.vector.tensor_scalar_mul(mcol[:, 0:nr], acol, rinv[:, :])
                    tmp = epool.tile([16, 16 * 32], F32, tag="etmp")
                    t3 = tmp[:, 0:nr * ncols].rearrange("p (j c) -> p j c", j=nr)
                    u = mcol[:, 0:nr].unsqueeze(2).broadcast_to([16, nr, ncols])
                    v = aaug[:, 32 * k + k + 1:32 * k + 32]
                    v = v.unsqueeze(1).broadcast_to([16, nr, ncols])
                    nc.vector.tensor_tensor(t3, u, v, mybir.AluOpType.mult)
                    blk = aaug[:, 32 * (k + 1):512]
                    blk = blk.rearrange("p (j c) -> p j c", c=32)[:, :, k + 1:32]
                    nc.vector.tensor_tensor(blk, blk, t3, mybir.AluOpType.subtract)

                # W rows: W[i, :] = sqrt(1/U[i,i]) * M[i, :]
                d16 = epool.tile([16, 16], F32, tag="d16")
                rsq = epool.tile([16, 16], F32, tag="rsq")
                nc.vector.reciprocal(d16[:, :], a_diag)
                nc.scalar.sqrt(rsq[:, :], d16[:, :])
                w3 = w_gl[:, :].rearrange("p (i k) -> p i k", i=16)
                nc.vector.tensor_tensor(
                    w3, m_part,
                    rsq[:, :].unsqueeze(2).broadcast_to([16, 16, 16]),
                    mybir.AluOpType.mult,
                )

            # scatter W into block-diagonal [128,128] halves (transposed:
            # W_bd[k_in, c_out] = W[c_out_local, k_local])
            nc.gpsimd.memset(w_bd0[:, :], 0.0)
            nc.gpsimd.memset(w_bd1[:, :], 0.0)
            nc.sync.dma_start(w_dram[:, :], w_gl[:, :])
            for g in range(G):
                h, gg = divmod(g, 8)
                dst_t = w_bd0 if h == 0 else w_bd1
                dst = dst_t[16 * gg:16 * gg + 16, 16 * gg:16 * gg + 16]
                src = w_dram[g].rearrange("(m k) -> k m", m=16)
                # SWDGE cast DMA fp32 -> bf16
                nc.gpsimd.dma_start(dst, src)

            # bias[c] = sum_k W[g(c), c_loc, k] mu[g(c), k], then replicate
            # to all 128 partitions via a K=1 ones matmul
            w3b = w_gl[:, :].rearrange("p (i k) -> p i k", i=16)
            wmu3 = wmu[:, :].rearrange("p (i k) -> p i k", i=16)
            nc.vector.tensor_tensor(
                wmu3, w3b,
                mu_gl[:, :].unsqueeze(1).broadcast_to([16, 16, 16]),
                mybir.AluOpType.mult,
            )
            nc.vector.reduce_sum(b_gl[:, :], wmu3, axis=mybir.AxisListType.X)
            # b_gl[g, m] -> flat channel order c = 16 g + m
            nc.sync.dma_start(
                b_dram[0].rearrange("(g m) -> g m", g=16), b_gl[:, :]
            )
            nc.sync.dma_start(b_row[0:1, :], b_dram[0:1, :])
            with tc.tile_pool(name="bpsum", bufs=1, space="PSUM") as bpool:
                brep = bpool.tile([128, 256], F32, tag="brep")
                nc.tensor.matmul(brep[:, :], ones1[0:1, :], b_row[0:1, :])
                nc.vector.tensor_copy(bias_rep[:, :], brep[:, :])

            # ---------------- phase 2: whiten + writeback ---------------
            with (
                tc.tile_pool(name="ostage", bufs=3) as opool,
                tc.tile_pool(name="popsum", bufs=3, space="PSUM") as popool,
            ):
                bias_v = bias_rep[:, :].rearrange("p (h c) -> p h c", h=2)
                for t in range(STAGES):
                    ost = opool.tile([128, JPER * 256], F32, tag="ost")
                    ost3 = ost[:, :].rearrange("p (j c) -> p j c", j=JPER)
                    for j in range(JPER):
                        ci = t * JPER + j
                        cs = slice(ci * CHUNK, (ci + 1) * CHUNK)
                        po = popool.tile([128, 1024], F32, tag="po")
                        nc.tensor.matmul(po[:, 0:128], x_cm0[:, cs], w_bd0[:, :])
                        nc.tensor.matmul(po[:, 512:640], x_cm1[:, cs], w_bd1[:, :])
                        po_v = po[:, :].rearrange("p (h c) -> p h c", h=2)
                        po_v = po_v[:, :, 0:128]
                        dst = ost3[:, j, :].rearrange("p (h c) -> p h c", h=2)
                        nc.vector.tensor_tensor(
                            dst, po_v, bias_v, mybir.AluOpType.subtract
                        )
                    dsto = o[t * JPER * CHUNK:(t + 1) * JPER * CHUNK, :]
                    dsto = dsto.rearrange("(j p) c -> p j c", j=JPER)
                    nc.sync.dma_start(dsto, ost3[:, :, :])

    nc.compile()
    return nc


def kernel(x: np.ndarray) -> np.ndarray:
    assert x.shape == (B, W_DIM, H_DIM, C) and x.dtype == np.float32
    if "nc" not in _CACHE:
        _CACHE["nc"] = _build()
    nc = _CACHE["nc"]
    xf = np.ascontiguousarray(x.reshape(N_TOT, C))
    in_maps = [
        {"x": xf[i * NL:(i + 1) * NL]} for i in range(NCORES)
    ]
    res = run_bass_kernel_spmd(nc, in_maps, list(range(NCORES)))
    out = np.concatenate([res.results[i]["o"] for i in range(NCORES)], axis=0)
    return out.reshape(B, W_DIM, H_DIM, C)


# revision 28
# speedup vs baseline: 1.3097x; 1.0136x over previous
"""Decorrelation (whitening) normalization on 8 Trainium2 NeuronCores.

Reference semantics (NHWC input x: [32, 64, 64, 256], fp32):
  flatten to [N=131072, C=256]; per-channel mean; per-group (16 groups of
  m=16 channels) covariance with shrinkage ((1-eps)*cov + eps*I)/denom
  (cov itself already divided by denom once); L = cholesky(cov);
  out = L^{-1} (x - mean) per group, back in NHWC layout.

Strategy (data-parallel over batch, 8 cores, full I/O):
  - each core takes a contiguous 16384-row slice of [131072, 256]
  - phase 1: stream 128-sample chunks; accumulate full Gram (augmented with
    a ones column -> channel sums) via f32r matmuls in PSUM; PE-transpose
    each chunk into a persistent channel-major SBUF copy of the slice
  - tiny AllReduce (17 KB) of block-diag Gram + sums across the 8 cores
  - on-device per-group LDL elimination on [A | I] -> L^{-1} = D^{-1/2} M
  - phase 2: whitening matmuls (x_cm chunk stationary, block-diag W moving)
    produce sample-major output in PSUM; bias (W @ mean) subtract fused into
    the PSUM->SBUF copy; contiguous DMA back to HBM.
"""

import numpy as np

import concourse.bacc as bacc
import concourse.mybir as mybir
from concourse import masks, tile
from concourse.bass_types import AP
from concourse.bass_utils import run_bass_kernel_spmd

NCORES = 8
B, W_DIM, H_DIM, C = 32, 64, 64, 256
N_TOT = B * W_DIM * H_DIM            # 131072 samples (global)
NL = N_TOT // NCORES                 # 16384 samples per core
G = 16                               # channel groups
M = 16                               # channels per group
EPS = 1e-3
DENOM = float(N_TOT - 1)

CHUNK = 128                          # samples per PE chunk
CHUNKS = NL // CHUNK                 # 128
JPER = 4                             # chunks per staging tile / DMA
STAGES = CHUNKS // JPER              # 32

F32 = mybir.dt.float32
BF16 = mybir.dt.bfloat16

# cov_final = A_COEF * Gram_raw - (A_COEF / N_TOT) * (S x S) + G_COEF * I
A_COEF = (1.0 - EPS) / (DENOM * DENOM)
G_COEF = EPS / DENOM

# group order used on the solve partitions: pi = 8h + 4o + q holds group
# g = 8h + 2q + o, which makes every post-allreduce gather DMA uniform
PERM = [8 * (p // 8) + 2 * (p % 4) + (p % 8) // 4 for p in range(16)]

_CACHE = {}


def _build():
    nc = bacc.Bacc(
        "TRN2", target_bir_lowering=False, debug=False, num_devices=NCORES
    )
    x = nc.dram_tensor("x", [NL, C], F32, kind="ExternalInput").ap()
    o = nc.dram_tensor("o", [NL, C], F32, kind="ExternalOutput").ap()

    with tile.TileContext(nc) as tc:
        with (
            tc.tile_pool(name="const", bufs=1) as cpool,
            tc.tile_pool(name="persist", bufs=1) as ppool,
            tc.tile_pool(name="dram", bufs=1, space="DRAM") as dpool,
        ):
            ident = cpool.tile([128, 128], BF16)
            masks.make_identity(nc, ident[:, :])
            ones1 = cpool.tile([1, 128], F32)
            nc.gpsimd.memset(ones1[:, :], 1.0)

            # channel-major bf16 copy of the local slice: half h holds
            # channels [128h, 128h+128) as [128, NL]
            x_cm0 = ppool.tile([128, NL], BF16)
            x_cm1 = ppool.tile([128, NL], BF16)
            w_bd0 = ppool.tile([128, 128], BF16)
            w_bd1 = ppool.tile([128, 128], BF16)
            nc.gpsimd.memset(w_bd0[:, :], 0.0)
            nc.gpsimd.memset(w_bd1[:, :], 0.0)
            # stats row p: [0:32] Gram diag stripe of ch p (half 0),
            # [32:64] same for ch 128+p, [64] sum ch p, [65] sum ch 128+p
            stats = ppool.tile([128, 66], F32)
            cc_in = dpool.tile([128, 66], F32)
            cc_out = dpool.tile([128, 66], F32)
            w_dram = dpool.tile([16, 256], BF16)
            b_dram = dpool.tile([1, 256], F32)

            # ---------------- phase 1: Gram + sums + transpose ----------
            with (
                tc.tile_pool(name="stage", bufs=4) as spool,
                tc.tile_pool(name="gpsum", bufs=1, space="PSUM") as gpool,
                tc.tile_pool(name="ptpsum", bufs=2, space="PSUM") as ptpool,
            ):
                g_top = gpool.tile([128, 257], F32, tag="gt")
                g_bot = gpool.tile([128, 257], F32, tag="gb")
                for t in range(STAGES):
                    # SWDGE casting DMA: fp32 HBM -> bf16 SBUF in flight
                    st = spool.tile([128, JPER * 257], BF16, tag="st")
                    st3 = st[:, :].rearrange("p (j c) -> p j c", j=JPER)
                    src = x[t * JPER * CHUNK:(t + 1) * JPER * CHUNK, :]
                    src = src.rearrange("(j p) c -> p j c", j=JPER)
                    nc.gpsimd.dma_start(st3[:, :, 0:256], src)
                    nc.gpsimd.memset(st3[:, :, 256], 1.0)
                    for j in range(JPER):
                        ci = t * JPER + j
                        chunk = st3[:, j, :]
                        first = ci == 0
                        last = ci == CHUNKS - 1
                        mov = chunk[:, 0:257]
                        nc.tensor.matmul(
                            g_top[:, :], chunk[:, 0:128], mov,
                            start=first, stop=last,
                        )
                        nc.tensor.matmul(
                            g_bot[:, :], chunk[:, 128:256], mov,
                            start=first, stop=last,
                        )
                        pt0 = ptpool.tile([128, 128], BF16, tag="pt0")
                        pt1 = ptpool.tile([128, 128], BF16, tag="pt1")
                        nc.tensor.transpose(pt0[:, :], chunk[:, 0:128], ident[:, :])
                        nc.tensor.transpose(pt1[:, :], chunk[:, 128:256], ident[:, :])
                        cs = slice(ci * CHUNK, (ci + 1) * CHUNK)
                        nc.vector.tensor_copy(x_cm0[:, cs], pt0[:, :])
                        nc.vector.tensor_copy(x_cm1[:, cs], pt1[:, :])

                # ship the 32-aligned diagonal stripes (hold all 16x16
                # group blocks) + the sums columns
                for q in range(4):
                    ps = slice(32 * q, 32 * q + 32)
                    nc.vector.tensor_copy(stats[ps, 0:32], g_top[ps, 32 * q:32 * q + 32])
                    nc.vector.tensor_copy(
                        stats[ps, 32:64], g_bot[ps, 128 + 32 * q:128 + 32 * q + 32]
                    )
                nc.vector.tensor_copy(stats[:, 64:65], g_top[:, 256:257])
                nc.vector.tensor_copy(stats[:, 65:66], g_bot[:, 256:257])

            # ---------------- all-reduce the stats ----------------------
            nc.sync.dma_start(cc_in[:, :], stats[:, :])
            nc.gpsimd.collective_compute(
                "AllReduce",
                mybir.AluOpType.add,
                replica_groups=[list(range(NCORES))],
                ins=[cc_in.opt()],
                outs=[cc_out.opt()],
            )

            # ---------------- tiny solve: W = L^{-1} per group ----------
            # Aaug [16 groups (part), 16 rows x (16 A cols + 16 M cols)]
            aaug = ppool.tile([16, 512], F32)
            s_gl = ppool.tile([16, 16], F32)
            mu_gl = ppool.tile([16, 16], F32)
            outer = ppool.tile([16, 256], F32)
            wmu = ppool.tile([16, 256], F32)
            b_gl = ppool.tile([16, 16], F32)
            w_gl = ppool.tile([16, 256], F32)
            b_row = ppool.tile([1, 256], F32)
            bias_rep = ppool.tile([128, 256], F32)

            a3 = aaug[:, :].rearrange("p (i c) -> p i c", i=16)   # [16,16,32]
            a_part = a3[:, :, 0:16]
            m_part = a3[:, :, 16:32]
            a_diag = aaug[:, 0:512:33]                            # A[i,i]
            m_diag = aaug[:, 16:512:33]                           # M[i,i]

            # gather per-group 16x16 Gram blocks + sums out of the
            # all-reduced stripes; solve partition pi holds group PERM[pi].
            # flat(p, c) = 66 p + c; block (g,i,k): row 32q+16o+i,
            # col 32h+16o+k with g = 8h+2q+o
            cc_t = cc_out[:, :].tensor
            for h in range(2):
                for par in range(2):
                    base = slice(8 * h + 4 * par, 8 * h + 4 * par + 4)
                    src = AP(
                        cc_t, 66 * 16 * par + 32 * h + 16 * par,
                        [[66 * 32, 4], [66, 16], [1, 16]],
                    )
                    nc.sync.dma_start(a3[base, :, 0:16], src)
                    src_s = AP(
                        cc_t, 66 * 16 * par + 64 + h, [[66 * 32, 4], [66, 16]]
                    )
                    nc.sync.dma_start(s_gl[base, :], src_s)
            nc.vector.tensor_scalar_mul(mu_gl[:, :], s_gl[:, :], 1.0 / N_TOT)

            # cov = A_COEF*Gram - (A_COEF/N)*S⊗S + G_COEF*I
            o3 = outer[:, :].rearrange("p (i k) -> p i k", i=16)
            nc.vector.tensor_tensor(
                o3,
                s_gl[:, :].unsqueeze(2).broadcast_to([16, 16, 16]),
                s_gl[:, :].unsqueeze(1).broadcast_to([16, 16, 16]),
                mybir.AluOpType.mult,
            )
            nc.vector.tensor_scalar_mul(o3, o3, A_COEF / N_TOT)
            nc.vector.tensor_scalar_mul(a_part, a_part, A_COEF)
            nc.vector.tensor_tensor(a_part, a_part, o3, mybir.AluOpType.subtract)
            nc.vector.tensor_scalar_add(a_diag, a_diag, G_COEF)
            # M := I
            nc.vector.memset(m_part, 0.0)
            nc.vector.memset(m_diag, 1.0)

            # gaussian elimination on [A | M]; row k eliminates rows k+1..15
            with tc.tile_pool(name="elim", bufs=2) as epool:
                for k in range(15):
                    nr = 15 - k          # rows below pivot
                    ncols = 31 - k       # remaining cols (A: k+1..15, M: 16)
                    rinv = epool.tile([16, 1], F32, tag="rinv")
                    nc.vector.reciprocal(rinv[:, :], aaug[:, 33 * k:33 * k + 1])
                    mcol = epool.tile([16, 16], F32, tag="mcol")
                    acol = aaug[:, 32 * (k + 1) + k:512:32]      # [16, nr]
                    nc.vector.tensor_scalar_mul(mcol[:, 0:nr], acol, rinv[:, :])
                    tmp = epool.tile([16, 16 * 32], F32, tag="etmp")
                    t3 = tmp[:, 0:nr * ncols].rearrange("p (j c) -> p j c", j=nr)
                    u = mcol[:, 0:nr].unsqueeze(2).broadcast_to([16, nr, ncols])
                    v = aaug[:, 32 * k + k + 1:32 * k + 32]
                    v = v.unsqueeze(1).broadcast_to([16, nr, ncols])
                    nc.vector.tensor_tensor(t3, u, v, mybir.AluOpType.mult)
                    blk = aaug[:, 32 * (k + 1):512]
                    blk = blk.rearrange("p (j c) -> p j c", c=32)[:, :, k + 1:32]
                    nc.vector.tensor_tensor(blk, blk, t3, mybir.AluOpType.subtract)

                # W rows: W[i, :] = sqrt(1/U[i,i]) * M[i, :]
                d16 = epool.tile([16, 16], F32, tag="d16")
                rsq = epool.tile([16, 16], F32, tag="rsq")
                nc.vector.reciprocal(d16[:, :], a_diag)
                nc.scalar.sqrt(rsq[:, :], d16[:, :])
                w3 = w_gl[:, :].rearrange("p (i k) -> p i k", i=16)
                nc.vector.tensor_tensor(
                    w3, m_part,
                    rsq[:, :].unsqueeze(2).broadcast_to([16, 16, 16]),
                    mybir.AluOpType.mult,
                )

            # scatter W into block-diagonal [128,128] halves (transposed:
            # W_bd[k_in, c_out] = W[c_out_local, k_local])
            # one casting DMA f32->bf16 to DRAM, then fast HWDGE scatter
            # loads into the block-diagonal positions (pi holds PERM[pi])
            nc.gpsimd.dma_start(w_dram[:, :], w_gl[:, :])
            for pi in range(G):
                g = PERM[pi]
                h, gg = divmod(g, 8)
                dst_t = w_bd0 if h == 0 else w_bd1
                dst = dst_t[16 * gg:16 * gg + 16, 16 * gg:16 * gg + 16]
                src = w_dram[pi].rearrange("(m k) -> k m", m=16)
                nc.sync.dma_start(dst, src)

            # bias[c] = sum_k W[g(c), c_loc, k] mu[g(c), k], then replicate
            # to all 128 partitions via a K=1 ones matmul
            w3b = w_gl[:, :].rearrange("p (i k) -> p i k", i=16)
            wmu3 = wmu[:, :].rearrange("p (i k) -> p i k", i=16)
            nc.vector.tensor_tensor(
                wmu3, w3b,
                mu_gl[:, :].unsqueeze(1).broadcast_to([16, 16, 16]),
                mybir.AluOpType.mult,
            )
            nc.vector.reduce_sum(b_gl[:, :], wmu3, axis=mybir.AxisListType.X)
            # b_gl[pi, m] -> flat channel c = 16 PERM[pi] + m: per (h, o)
            # quartet the dram offsets are uniform (step 32 over q)
            b_t = b_dram[0:1, :].tensor
            for h in range(2):
                for par in range(2):
                    src_b = b_gl[8 * h + 4 * par:8 * h + 4 * par + 4, :]
                    dst_b = AP(b_t, 16 * (8 * h + par), [[32, 4], [1, 16]])
                    nc.sync.dma_start(dst_b, src_b)
            nc.sync.dma_start(b_row[0:1, :], b_dram[0:1, :])
            with tc.tile_pool(name="bpsum", bufs=1, space="PSUM") as bpool:
                brep = bpool.tile([128, 256], F32, tag="brep")
                nc.tensor.matmul(brep[:, :], ones1[0:1, :], b_row[0:1, :])
                nc.vector.tensor_copy(bias_rep[:, :], brep[:, :])

            # ---------------- phase 2: whiten + writeback ---------------
            with (
                tc.tile_pool(name="ostage", bufs=4) as opool,
                tc.tile_pool(name="popsum", bufs=4, space="PSUM") as popool,
            ):
                bias_v = bias_rep[:, :].rearrange("p (h c) -> p h c", h=2)
                for t in range(STAGES):
                    ost = opool.tile([128, JPER * 256], F32, tag="ost")
                    ost3 = ost[:, :].rearrange("p (j c) -> p j c", j=JPER)
                    for j in range(JPER):
                        ci = t * JPER + j
                        cs = slice(ci * CHUNK, (ci + 1) * CHUNK)
                        po = popool.tile([128, 1024], F32, tag="po")
                        nc.tensor.matmul(po[:, 0:128], x_cm0[:, cs], w_bd0[:, :])
                        nc.tensor.matmul(po[:, 512:640], x_cm1[:, cs], w_bd1[:, :])
                        po_v = po[:, :].rearrange("p (h c) -> p h c", h=2)
                        po_v = po_v[:, :, 0:128]
                        dst = ost3[:, j, :].rearrange("p (h c) -> p h c", h=2)
                        nc.vector.tensor_tensor(
                            dst, po_v, bias_v, mybir.AluOpType.subtract
                        )
                    dsto = o[t * JPER * CHUNK:(t + 1) * JPER * CHUNK, :]
                    dsto = dsto.rearrange("(j p) c -> p j c", j=JPER)
                    nc.sync.dma_start(dsto, ost3[:, :, :])

    nc.compile()
    return nc


def kernel(x: np.ndarray) -> np.ndarray:
    assert x.shape == (B, W_DIM, H_DIM, C) and x.dtype == np.float32
    if "nc" not in _CACHE:
        _CACHE["nc"] = _build()
    nc = _CACHE["nc"]
    xf = np.ascontiguousarray(x.reshape(N_TOT, C))
    in_maps = [
        {"x": xf[i * NL:(i + 1) * NL]} for i in range(NCORES)
    ]
    res = run_bass_kernel_spmd(nc, in_maps, list(range(NCORES)))
    out = np.concatenate([res.results[i]["o"] for i in range(NCORES)], axis=0)
    return out.reshape(B, W_DIM, H_DIM, C)


# revision 31
# speedup vs baseline: 1.4185x; 1.0831x over previous
"""Decorrelation (whitening) normalization on 8 Trainium2 NeuronCores.

Reference semantics (NHWC input x: [32, 64, 64, 256], fp32):
  flatten to [N=131072, C=256]; per-channel mean; per-group (16 groups of
  m=16 channels) covariance with shrinkage ((1-eps)*cov + eps*I)/denom
  (cov itself already divided by denom once); L = cholesky(cov);
  out = L^{-1} (x - mean) per group, back in NHWC layout.

Strategy (data-parallel over batch, 8 cores, full I/O):
  - each core takes a contiguous 16384-row slice of [131072, 256]
  - phase 1: stream 128-sample chunks; accumulate full Gram (augmented with
    a ones column -> channel sums) via f32r matmuls in PSUM; PE-transpose
    each chunk into a persistent channel-major SBUF copy of the slice
  - tiny AllReduce (17 KB) of block-diag Gram + sums across the 8 cores
  - on-device per-group LDL elimination on [A | I] -> L^{-1} = D^{-1/2} M
  - phase 2: whitening matmuls (x_cm chunk stationary, block-diag W moving)
    produce sample-major output in PSUM; bias (W @ mean) subtract fused into
    the PSUM->SBUF copy; contiguous DMA back to HBM.
"""

import numpy as np

import concourse.bacc as bacc
import concourse.mybir as mybir
from concourse import masks, tile
from concourse.bass_types import AP
from concourse.bass_utils import run_bass_kernel_spmd

NCORES = 8
B, W_DIM, H_DIM, C = 32, 64, 64, 256
N_TOT = B * W_DIM * H_DIM            # 131072 samples (global)
NL = N_TOT // NCORES                 # 16384 samples per core
G = 16                               # channel groups
M = 16                               # channels per group
EPS = 1e-3
DENOM = float(N_TOT - 1)

CHUNK = 128                          # samples per PE chunk
CHUNKS = NL // CHUNK                 # 128
JPER = 4                             # chunks per staging tile / DMA
STAGES = CHUNKS // JPER              # 32

F32 = mybir.dt.float32
BF16 = mybir.dt.bfloat16

# cov_final = A_COEF * Gram_raw - (A_COEF / N_TOT) * (S x S) + G_COEF * I
A_COEF = (1.0 - EPS) / (DENOM * DENOM)
G_COEF = EPS / DENOM

# group order used on the solve partitions: pi = 8h + 4o + q holds group
# g = 8h + 2q + o, which makes every post-allreduce gather DMA uniform
PERM = [8 * (p // 8) + 2 * (p % 4) + (p % 8) // 4 for p in range(16)]

_CACHE = {}


def _build():
    nc = bacc.Bacc(
        "TRN2", target_bir_lowering=False, debug=False, num_devices=NCORES
    )
    x = nc.dram_tensor("x", [NL, C], F32, kind="ExternalInput").ap()
    o = nc.dram_tensor("o", [NL, C], F32, kind="ExternalOutput").ap()

    with tile.TileContext(nc) as tc:
        with (
            tc.tile_pool(name="const", bufs=1) as cpool,
            tc.tile_pool(name="persist", bufs=1) as ppool,
            tc.tile_pool(name="dram", bufs=1, space="DRAM") as dpool,
        ):
            ident = cpool.tile([128, 128], BF16)
            masks.make_identity(nc, ident[:, :])
            ones1 = cpool.tile([1, 128], F32)
            nc.gpsimd.memset(ones1[:, :], 1.0)

            # channel-major bf16 copy of the local slice: half h holds
            # channels [128h, 128h+128) as [128, NL]
            x_cm0 = ppool.tile([128, NL], BF16)
            x_cm1 = ppool.tile([128, NL], BF16)
            w_bd0 = ppool.tile([128, 128], BF16)
            w_bd1 = ppool.tile([128, 128], BF16)
            nc.gpsimd.memset(w_bd0[:, :], 0.0)
            nc.gpsimd.memset(w_bd1[:, :], 0.0)
            # stats row p: [0:32] Gram diag stripe of ch p (half 0),
            # [32:64] same for ch 128+p, [64] sum ch p, [65] sum ch 128+p
            stats = ppool.tile([128, 66], F32)
            cc_in = dpool.tile([128, 66], F32)
            cc_out = dpool.tile([128, 66], F32)
            w_dram = dpool.tile([16, 256], BF16)
            b_dram = dpool.tile([1, 256], F32)

            # ---------------- phase 1: Gram + sums + transpose ----------
            with (
                tc.tile_pool(name="stage", bufs=4) as spool,
                tc.tile_pool(name="gpsum", bufs=1, space="PSUM") as gpool,
                tc.tile_pool(name="ptpsum", bufs=2, space="PSUM") as ptpool,
            ):
                # stage chunk layout: [x_lo (128) | ones (1) | pad (1) |
                # x_hi (128)] = 258 cols. Each Gram half only computes its
                # own 128-col block (+ sums); every stationary operand
                # starts at an even (4-byte aligned) bf16 element offset.
                g_top = gpool.tile([128, 129], F32, tag="gt")
                g_bot = gpool.tile([128, 130], F32, tag="gb")
                for t in range(STAGES):
                    # SWDGE casting DMAs: fp32 HBM -> bf16 SBUF in flight
                    st = spool.tile([128, JPER * 258], BF16, tag="st")
                    st3 = st[:, :].rearrange("p (j c) -> p j c", j=JPER)
                    src = x[t * JPER * CHUNK:(t + 1) * JPER * CHUNK, :]
                    src = src.rearrange("(j p) c -> p j c", j=JPER)
                    nc.gpsimd.dma_start(st3[:, :, 0:128], src[:, :, 0:128])
                    nc.gpsimd.dma_start(st3[:, :, 130:258], src[:, :, 128:256])
                    nc.gpsimd.memset(st3[:, :, 128:130], 1.0)
                    for j in range(JPER):
                        ci = t * JPER + j
                        chunk = st3[:, j, :]
                        first = ci == 0
                        last = ci == CHUNKS - 1
                        nc.tensor.matmul(
                            g_top[:, :], chunk[:, 0:128], chunk[:, 0:129],
                            start=first, stop=last,
                        )
                        # rhs col 0 = ones (sums), col 1 = pad (ignored)
                        nc.tensor.matmul(
                            g_bot[:, :], chunk[:, 130:258], chunk[:, 128:258],
                            start=first, stop=last,
                        )
                        pt0 = ptpool.tile([128, 128], BF16, tag="pt0")
                        pt1 = ptpool.tile([128, 128], BF16, tag="pt1")
                        nc.tensor.transpose(pt0[:, :], chunk[:, 0:128], ident[:, :])
                        nc.tensor.transpose(pt1[:, :], chunk[:, 130:258], ident[:, :])
                        cs = slice(ci * CHUNK, (ci + 1) * CHUNK)
                        nc.vector.tensor_copy(x_cm0[:, cs], pt0[:, :])
                        nc.vector.tensor_copy(x_cm1[:, cs], pt1[:, :])

                # ship the 32-aligned diagonal stripes (hold all 16x16
                # group blocks) + the sums columns
                for q in range(4):
                    ps = slice(32 * q, 32 * q + 32)
                    nc.vector.tensor_copy(stats[ps, 0:32], g_top[ps, 32 * q:32 * q + 32])
                    nc.vector.tensor_copy(
                        stats[ps, 32:64], g_bot[ps, 2 + 32 * q:2 + 32 * q + 32]
                    )
                nc.vector.tensor_copy(stats[:, 64:65], g_top[:, 128:129])
                nc.vector.tensor_copy(stats[:, 65:66], g_bot[:, 0:1])

            # ---------------- all-reduce the stats ----------------------
            nc.sync.dma_start(cc_in[:, :], stats[:, :])
            nc.gpsimd.collective_compute(
                "AllReduce",
                mybir.AluOpType.add,
                replica_groups=[list(range(NCORES))],
                ins=[cc_in.opt()],
                outs=[cc_out.opt()],
            )

            # ---------------- tiny solve: W = L^{-1} per group ----------
            # Aaug [16 groups (part), 16 rows x (16 A cols + 16 M cols)]
            aaug = ppool.tile([16, 512], F32)
            s_gl = ppool.tile([16, 16], F32)
            mu_gl = ppool.tile([16, 16], F32)
            outer = ppool.tile([16, 256], F32)
            wmu = ppool.tile([16, 256], F32)
            b_gl = ppool.tile([16, 16], F32)
            w_gl = ppool.tile([16, 256], F32)
            b_row = ppool.tile([1, 256], F32)
            bias_rep = ppool.tile([128, 256], F32)

            a3 = aaug[:, :].rearrange("p (i c) -> p i c", i=16)   # [16,16,32]
            a_part = a3[:, :, 0:16]
            m_part = a3[:, :, 16:32]
            a_diag = aaug[:, 0:512:33]                            # A[i,i]
            m_diag = aaug[:, 16:512:33]                           # M[i,i]

            # gather per-group 16x16 Gram blocks + sums out of the
            # all-reduced stripes; solve partition pi holds group PERM[pi].
            # flat(p, c) = 66 p + c; block (g,i,k): row 32q+16o+i,
            # col 32h+16o+k with g = 8h+2q+o
            cc_t = cc_out[:, :].tensor
            for h in range(2):
                for par in range(2):
                    base = slice(8 * h + 4 * par, 8 * h + 4 * par + 4)
                    src = AP(
                        cc_t, 66 * 16 * par + 32 * h + 16 * par,
                        [[66 * 32, 4], [66, 16], [1, 16]],
                    )
                    nc.sync.dma_start(a3[base, :, 0:16], src)
                    src_s = AP(
                        cc_t, 66 * 16 * par + 64 + h, [[66 * 32, 4], [66, 16]]
                    )
                    nc.sync.dma_start(s_gl[base, :], src_s)
            nc.vector.tensor_scalar_mul(mu_gl[:, :], s_gl[:, :], 1.0 / N_TOT)

            # cov = A_COEF*Gram - (A_COEF/N)*S⊗S + G_COEF*I
            o3 = outer[:, :].rearrange("p (i k) -> p i k", i=16)
            nc.vector.tensor_tensor(
                o3,
                s_gl[:, :].unsqueeze(2).broadcast_to([16, 16, 16]),
                s_gl[:, :].unsqueeze(1).broadcast_to([16, 16, 16]),
                mybir.AluOpType.mult,
            )
            nc.vector.tensor_scalar_mul(o3, o3, A_COEF / N_TOT)
            nc.vector.tensor_scalar_mul(a_part, a_part, A_COEF)
            nc.vector.tensor_tensor(a_part, a_part, o3, mybir.AluOpType.subtract)
            nc.vector.tensor_scalar_add(a_diag, a_diag, G_COEF)
            # M := I
            nc.vector.memset(m_part, 0.0)
            nc.vector.memset(m_diag, 1.0)

            # gaussian elimination on [A | M]; row k eliminates rows k+1..15
            with tc.tile_pool(name="elim", bufs=2) as epool:
                for k in range(15):
                    nr = 15 - k          # rows below pivot
                    ncols = 31 - k       # remaining cols (A: k+1..15, M: 16)
                    rinv = epool.tile([16, 1], F32, tag="rinv")
                    nc.vector.reciprocal(rinv[:, :], aaug[:, 33 * k:33 * k + 1])
                    mcol = epool.tile([16, 16], F32, tag="mcol")
                    acol = aaug[:, 32 * (k + 1) + k:512:32]      # [16, nr]
                    nc.vector.tensor_scalar_mul(mcol[:, 0:nr], acol, rinv[:, :])
                    tmp = epool.tile([16, 16 * 32], F32, tag="etmp")
                    t3 = tmp[:, 0:nr * ncols].rearrange("p (j c) -> p j c", j=nr)
                    u = mcol[:, 0:nr].unsqueeze(2).broadcast_to([16, nr, ncols])
                    v = aaug[:, 32 * k + k + 1:32 * k + 32]
                    v = v.unsqueeze(1).broadcast_to([16, nr, ncols])
                    nc.vector.tensor_tensor(t3, u, v, mybir.AluOpType.mult)
                    blk = aaug[:, 32 * (k + 1):512]
                    blk = blk.rearrange("p (j c) -> p j c", c=32)[:, :, k + 1:32]
                    nc.vector.tensor_tensor(blk, blk, t3, mybir.AluOpType.subtract)

                # W rows: W[i, :] = sqrt(1/U[i,i]) * M[i, :]
                d16 = epool.tile([16, 16], F32, tag="d16")
                rsq = epool.tile([16, 16], F32, tag="rsq")
                nc.vector.reciprocal(d16[:, :], a_diag)
                nc.scalar.sqrt(rsq[:, :], d16[:, :])
                w3 = w_gl[:, :].rearrange("p (i k) -> p i k", i=16)
                nc.vector.tensor_tensor(
                    w3, m_part,
                    rsq[:, :].unsqueeze(2).broadcast_to([16, 16, 16]),
                    mybir.AluOpType.mult,
                )

            # scatter W into block-diagonal [128,128] halves (transposed:
            # W_bd[k_in, c_out] = W[c_out_local, k_local])
            # one casting DMA f32->bf16 to DRAM, then fast HWDGE scatter
            # loads into the block-diagonal positions (pi holds PERM[pi])
            nc.gpsimd.dma_start(w_dram[:, :], w_gl[:, :])
            for pi in range(G):
                g = PERM[pi]
                h, gg = divmod(g, 8)
                dst_t = w_bd0 if h == 0 else w_bd1
                dst = dst_t[16 * gg:16 * gg + 16, 16 * gg:16 * gg + 16]
                src = w_dram[pi].rearrange("(m k) -> k m", m=16)
                # split issue across both HWDGE engines
                eng = nc.sync if pi % 2 == 0 else nc.scalar
                eng.dma_start(dst, src)

            # bias[c] = sum_k W[g(c), c_loc, k] mu[g(c), k], then replicate
            # to all 128 partitions via a K=1 ones matmul
            w3b = w_gl[:, :].rearrange("p (i k) -> p i k", i=16)
            wmu3 = wmu[:, :].rearrange("p (i k) -> p i k", i=16)
            nc.vector.tensor_tensor(
                wmu3, w3b,
                mu_gl[:, :].unsqueeze(1).broadcast_to([16, 16, 16]),
                mybir.AluOpType.mult,
            )
            nc.vector.reduce_sum(b_gl[:, :], wmu3, axis=mybir.AxisListType.X)
            # b_gl[pi, m] -> flat channel c = 16 PERM[pi] + m: per (h, o)
            # quartet the dram offsets are uniform (step 32 over q)
            b_t = b_dram[0:1, :].tensor
            for h in range(2):
                for par in range(2):
                    src_b = b_gl[8 * h + 4 * par:8 * h + 4 * par + 4, :]
                    dst_b = AP(b_t, 16 * (8 * h + par), [[32, 4], [1, 16]])
                    nc.sync.dma_start(dst_b, src_b)
            nc.sync.dma_start(b_row[0:1, :], b_dram[0:1, :])
            with tc.tile_pool(name="bpsum", bufs=1, space="PSUM") as bpool:
                brep = bpool.tile([128, 256], F32, tag="brep")
                nc.tensor.matmul(brep[:, :], ones1[0:1, :], b_row[0:1, :])
                nc.vector.tensor_copy(bias_rep[:, :], brep[:, :])

            # ---------------- phase 2: whiten + writeback ---------------
            with (
                tc.tile_pool(name="ostage", bufs=4) as opool,
                tc.tile_pool(name="popsum", bufs=4, space="PSUM") as popool,
            ):
                bias_v = bias_rep[:, :].rearrange("p (h c) -> p h c", h=2)
                for t in range(STAGES):
                    ost = opool.tile([128, JPER * 256], F32, tag="ost")
                    ost3 = ost[:, :].rearrange("p (j c) -> p j c", j=JPER)
                    for j in range(JPER):
                        ci = t * JPER + j
                        cs = slice(ci * CHUNK, (ci + 1) * CHUNK)
                        po = popool.tile([128, 1024], F32, tag="po")
                        nc.tensor.matmul(po[:, 0:128], x_cm0[:, cs], w_bd0[:, :])
                        nc.tensor.matmul(po[:, 512:640], x_cm1[:, cs], w_bd1[:, :])
                        po_v = po[:, :].rearrange("p (h c) -> p h c", h=2)
                        po_v = po_v[:, :, 0:128]
                        dst = ost3[:, j, :].rearrange("p (h c) -> p h c", h=2)
                        nc.vector.tensor_tensor(
                            dst, po_v, bias_v, mybir.AluOpType.subtract
                        )
                    dsto = o[t * JPER * CHUNK:(t + 1) * JPER * CHUNK, :]
                    dsto = dsto.rearrange("(j p) c -> p j c", j=JPER)
                    nc.sync.dma_start(dsto, ost3[:, :, :])

    nc.compile()
    return nc


def kernel(x: np.ndarray) -> np.ndarray:
    assert x.shape == (B, W_DIM, H_DIM, C) and x.dtype == np.float32
    if "nc" not in _CACHE:
        _CACHE["nc"] = _build()
    nc = _CACHE["nc"]
    xf = np.ascontiguousarray(x.reshape(N_TOT, C))
    in_maps = [
        {"x": xf[i * NL:(i + 1) * NL]} for i in range(NCORES)
    ]
    res = run_bass_kernel_spmd(nc, in_maps, list(range(NCORES)))
    out = np.concatenate([res.results[i]["o"] for i in range(NCORES)], axis=0)
    return out.reshape(B, W_DIM, H_DIM, C)
